# revision 14
# baseline (speedup 1.0000x reference)
"""Cross-attention Trainium2 kernel (B=8, T=1024, S=1500, D=1024, H=16, Dh=64).

Sharding: pure data-parallel on batch — core b computes batch b end to end
(no collectives). Per-core pipeline, all operands SBUF-resident in bf16:

  qT[e,t]   = (wq*s).T^T @ x.T      (PE, contraction d, out [e,t])
  scT[s,t]  = kT_h^T @ qT_h         (PE, K=Dh=64; two heads packed in the
                                     128-row array via partition bases 0/64)
  PT[s,t]   = exp(scT)              (ACT, PSUM->SBUF bf16; no max-subtraction:
                                     scores are O(1) for this problem)
  pv[dh,t]  = vaug_h^T @ PT_h       (PE, vaug has a ones column so row 64
                                     accumulates the softmax denominator)
  attnT     = pv[0:64] * (1/pv[64]) (DVE + one SB->SB broadcast DMA)
  outT[f,t] = woT^T @ attnT + bo    (PE + DVE)

Host side pre-transposes/casts inputs and transposes the [f,t] output back.
"""

import sys

for _p in ("/opt/trn_rl_repo", "/root/.axon_site/_ro/trn_rl_repo"):
    if _p not in sys.path:
        sys.path.insert(0, _p)

import numpy as np
import ml_dtypes

import concourse.bass as bass
import concourse.mybir as mybir
import concourse.tile as tile
from concourse import bacc
from concourse import bass_utils

BF16 = ml_dtypes.bfloat16

P = 128
B = 8
T = 1024
S0 = 1500          # real source length
S = 1536           # padded to 12*128
D = 1024
H = 16
Dh = 64
DT = D // P        # 8 d/e/f tiles
ST = S // P        # 12 s chunks
NPAIR = H // 2     # 8 head pairs
HW = Dh + 1        # 65: per-head v width incl. ones column
SCALE = Dh ** -0.5

f32 = mybir.dt.float32
bf16 = mybir.dt.bfloat16


def build_bass():
    nc = bacc.Bacc("TRN2", target_bir_lowering=False, debug=False,
                   enable_asserts=False, num_devices=B)

    xT_d = nc.dram_tensor("xT", [D, T], bf16, kind="ExternalInput")
    kT_d = nc.dram_tensor("kT", [D, S], bf16, kind="ExternalInput")
    va_d = nc.dram_tensor("vaug", [S, H * HW], bf16, kind="ExternalInput")
    wqT_d = nc.dram_tensor("wqT", [D, D], bf16, kind="ExternalInput")
    bq_d = nc.dram_tensor("bqr", [P, DT], f32, kind="ExternalInput")
    woT_d = nc.dram_tensor("woT", [D, D], bf16, kind="ExternalInput")
    bo_d = nc.dram_tensor("bor", [P, DT], f32, kind="ExternalInput")
    outT_d = nc.dram_tensor("outT", [D, T], f32, kind="ExternalOutput")

    EXP = mybir.ActivationFunctionType.Exp

    with tile.TileContext(nc) as tc:
        with (
            tc.tile_pool(name="const", bufs=1) as cp,
            tc.tile_pool(name="work", bufs=2) as wp,
            tc.tile_pool(name="psum_mm", bufs=2, space="PSUM") as mmp,
            tc.tile_pool(name="psum_pv", bufs=2, space="PSUM") as pvp,
        ):
            # ---- persistent SBUF loads -------------------------------------
            def load1(dram, cols, j, tagbase, dt=bf16):
                t = cp.tile([P, cols], dt, name=f"{tagbase}{j}",
                            tag=f"{tagbase}{j}")
                nc.sync.dma_start(t[:], dram[j * P:(j + 1) * P, :])
                return t

            bq_sb = cp.tile([P, DT], f32, name="bq_sb", tag="bq_sb")
            nc.sync.dma_start(bq_sb[:], bq_d[:, :])
            # order matters: q_proj needs all of xT+wqT; pair 0 additionally
            # needs kT[0] and the vaug chunks — issue those before the rest.
            xT_sb = [load1(xT_d, T, j, "xTs") for j in range(DT)]
            kT_sb = {0: load1(kT_d, S, 0, "kTs")}
            va_sb = {0: load1(va_d, H * HW, 0, "vas")}
            wqT_sb = [load1(wqT_d, D, j, "wqTs") for j in range(DT)]
            va_sb.update({c: load1(va_d, H * HW, c, "vas") for c in (1, 2, 3)})
            kT_sb.update({j: load1(kT_d, S, j, "kTs") for j in range(1, DT)})
            va_sb.update({c: load1(va_d, H * HW, c, "vas")
                          for c in range(4, ST)})
            woT_sb = [load1(woT_d, D, j, "woTs") for j in range(DT)]
            bo_sb = cp.tile([P, DT], f32, name="bo_sb", tag="bo_sb")
            nc.sync.dma_start(bo_sb[:], bo_d[:, :])

            qT_sb = [cp.tile([P, T], bf16, name=f"qTs{j}", tag=f"qTs{j}")
                     for j in range(DT)]
            aT_sb = [cp.tile([P, T], bf16, name=f"aTs{j}", tag=f"aTs{j}")
                     for j in range(DT)]

            # ---- phase 1: q projection  qT[e,t] ----------------------------
            def qproj(j):
                ps = mmp.tile([P, T], f32, name=f"qp{j}", tag="mm")
                for tch in range(2):
                    tsl = slice(tch * 512, (tch + 1) * 512)
                    for dt_i in range(DT):
                        nc.tensor.matmul(
                            ps[:, tsl],
                            lhsT=wqT_sb[dt_i][:, j * P:(j + 1) * P],
                            rhs=xT_sb[dt_i][:, tsl],
                            start=(dt_i == 0), stop=(dt_i == DT - 1),
                        )
                nc.vector.tensor_scalar_add(qT_sb[j][:, :], ps[:, :],
                                            bq_sb[:, j:j + 1])

            # ---- phase 2: attention, one head pair per e-tile --------------
            def attn_pair(j):
                pv = [pvp.tile([P, T], f32, name=f"pv{j}_{a}", tag="pv")
                      for a in range(2)]
                for c in range(ST):
                    csl = slice(c * P, (c + 1) * P)
                    # two K=64 score matmuls per head, heads at partition
                    # bases 0/64 -> disjoint PE row groups; emit adjacently
                    # so they execute concurrently in the array.
                    sc = [mmp.tile([P, T], f32, name=f"sc{j}_{c}_{a}",
                                   tag="mm") for a in range(2)]
                    for a in range(2):
                        rows = slice(a * Dh, (a + 1) * Dh)
                        for tch in range(2):
                            tsl = slice(tch * 512, (tch + 1) * 512)
                            nc.tensor.matmul(
                                sc[a][:, tsl],
                                lhsT=kT_sb[j][rows, csl],
                                rhs=qT_sb[j][rows, tsl],
                                start=True, stop=True,
                            )
                    pt = [None, None]
                    for a in range(2):
                        pt[a] = wp.tile([P, T], bf16, name=f"pt{j}_{c}_{a}",
                                        tag="pt", bufs=8)
                        nc.scalar.activation(pt[a][:, :], sc[a][:, :], EXP)
                    for a in range(2):
                        h = 2 * j + a
                        for tch in range(2):
                            tsl = slice(tch * 512, (tch + 1) * 512)
                            nc.tensor.matmul(
                                pv[a][0:HW, tsl],
                                lhsT=va_sb[c][:, h * HW:(h + 1) * HW],
                                rhs=pt[a][:, tsl],
                                start=(c == 0), stop=(c == ST - 1),
                            )
                # evict PSUM fast, then normalize out of SBUF -> attnT (bf16)
                for a in range(2):
                    pvsb = wp.tile([HW, T], f32, name=f"pvsb{j}_{a}",
                                   tag="pvsb", bufs=3)
                    nc.vector.tensor_copy(pvsb[:, :], pv[a][0:HW, :])
                    # reciprocal of the [1,1024] denom row is ~6.5us (one
                    # lane, multi-pass); reshape to [64,16] via SB->SB DMA
                    # so all 64 lanes work -> ~0.4us.
                    dsm = wp.tile([Dh, 16], f32, name=f"ds{j}_{a}",
                                  tag="dsm", bufs=4)
                    nc.sync.dma_start(dsm[:, :], pvsb[Dh:Dh + 1, :])
                    nc.vector.reciprocal(dsm[:, :], dsm[:, :])
                    rrow = wp.tile([1, T], f32, name=f"rr{j}_{a}", tag="rrow",
                                   bufs=4)
                    nc.sync.dma_start(rrow[:, :], dsm[:, :])
                    nrm = wp.tile([Dh, T], f32, name=f"nr{j}_{a}", tag="nrm",
                                  bufs=4)
                    nc.gpsimd.partition_broadcast(nrm[:, :], rrow[0:1, :])
                    nc.vector.tensor_mul(
                        aT_sb[j][a * Dh:(a + 1) * Dh, :],
                        pvsb[0:Dh, :], nrm[:, :])

            # pair 0 right after qT[0] so ACT starts ~30us earlier; the
            # remaining q_proj tiles run on PE inside pair 0's ACT window.
            qproj(0)
            attn_pair(0)
            for j in range(1, DT):
                qproj(j)
            for j in range(1, NPAIR):
                attn_pair(j)

            # ---- phase 3: out projection  outT[f,t] ------------------------
            for fj in range(DT):
                ps = mmp.tile([P, T], f32, name=f"op{fj}", tag="mm")
                for tch in range(2):
                    tsl = slice(tch * 512, (tch + 1) * 512)
                    for et in range(DT):
                        nc.tensor.matmul(
                            ps[:, tsl],
                            lhsT=woT_sb[et][:, fj * P:(fj + 1) * P],
                            rhs=aT_sb[et][:, tsl],
                            start=(et == 0), stop=(et == DT - 1),
                        )
                ost = wp.tile([P, T], f32, name=f"ost{fj}", tag="ost", bufs=3)
                nc.vector.tensor_scalar_add(ost[:, :], ps[:, :],
                                            bo_sb[:, fj:fj + 1])
                nc.sync.dma_start(outT_d[fj * P:(fj + 1) * P, :], ost[:, :])

    nc.compile()
    return nc


def prep_inputs(x, k, v, wq, bq, wo, bo):
    """Host-side shard + layout prep. Returns per-core in_maps."""
    x = np.asarray(x, np.float32)
    k = np.asarray(k, np.float32)
    v = np.asarray(v, np.float32)
    wq = np.asarray(wq, np.float32)
    bq = np.asarray(bq, np.float32)
    wo = np.asarray(wo, np.float32)
    bo = np.asarray(bo, np.float32)

    wqT = np.ascontiguousarray((wq * SCALE).T).astype(BF16)       # [d, e]
    bqr = np.ascontiguousarray((bq * SCALE).reshape(DT, P).T)     # [P, DT]
    woT = np.ascontiguousarray(wo.T).astype(BF16)                 # [e, f]
    bor = np.ascontiguousarray(bo.reshape(DT, P).T)               # [P, DT]

    in_maps = []
    for b in range(x.shape[0]):
        xT = np.ascontiguousarray(x[b].T).astype(BF16)            # [D, T]
        kT = np.zeros((D, S), BF16)
        kT[:, :S0] = k[b].T.astype(BF16)
        vaug = np.zeros((S, H * HW), BF16)
        vb = v[b].astype(BF16)
        for h in range(H):
            vaug[:S0, h * HW:h * HW + Dh] = vb[:, h * Dh:(h + 1) * Dh]
            vaug[:S0, h * HW + Dh] = BF16(1.0)
        in_maps.append({
            "xT": xT, "kT": kT, "vaug": np.ascontiguousarray(vaug),
            "wqT": wqT, "bqr": bqr, "woT": woT, "bor": bor,
        })
    return in_maps


_NC_CACHE = {}


def kernel(x, k, v, wq, bq, wo, bo, _trace=False):
    if "nc" not in _NC_CACHE:
        _NC_CACHE["nc"] = build_bass()
    nc = _NC_CACHE["nc"]
    in_maps = prep_inputs(x, k, v, wq, bq, wo, bo)
    res = bass_utils.run_bass_kernel_spmd(
        nc, in_maps, core_ids=list(range(B)), trace=_trace)
    _NC_CACHE["last_result"] = res
    out = np.stack([np.ascontiguousarray(r["outT"].T) for r in res.results])
    return out


# revision 15
# speedup vs baseline: 1.0827x; 1.0827x over previous
"""Cross-attention Trainium2 kernel (B=8, T=1024, S=1500, D=1024, H=16, Dh=64).

Sharding: pure data-parallel on batch — core b computes batch b end to end
(no collectives). Per-core pipeline, all operands SBUF-resident in bf16:

  qT[e,t]   = (wq*s).T^T @ x.T      (PE, contraction d, out [e,t])
  scT[s,t]  = kT_h^T @ qT_h         (PE, K=Dh=64; two heads packed in the
                                     128-row array via partition bases 0/64)
  PT[s,t]   = exp(scT)              (ACT, PSUM->SBUF bf16; no max-subtraction:
                                     scores are O(1) for this problem)
  pv[dh,t]  = vaug_h^T @ PT_h       (PE, vaug has a ones column so row 64
                                     accumulates the softmax denominator)
  attnT     = pv[0:64] * (1/pv[64]) (DVE + one SB->SB broadcast DMA)
  outT[f,t] = woT^T @ attnT + bo    (PE + DVE)

Host side pre-transposes/casts inputs and transposes the [f,t] output back.
"""

import sys

for _p in ("/opt/trn_rl_repo", "/root/.axon_site/_ro/trn_rl_repo"):
    if _p not in sys.path:
        sys.path.insert(0, _p)

import numpy as np
import ml_dtypes

import concourse.bass as bass
import concourse.mybir as mybir
import concourse.tile as tile
from concourse import bacc
from concourse import bass_utils

BF16 = ml_dtypes.bfloat16

P = 128
B = 8
T = 1024
S0 = 1500          # real source length
S = 1536           # padded to 12*128
D = 1024
H = 16
Dh = 64
DT = D // P        # 8 d/e/f tiles
ST = S // P        # 12 s chunks
NPAIR = H // 2     # 8 head pairs
HW = Dh + 1        # 65: per-head v width incl. ones column
SCALE = Dh ** -0.5

f32 = mybir.dt.float32
bf16 = mybir.dt.bfloat16


def build_bass():
    nc = bacc.Bacc("TRN2", target_bir_lowering=False, debug=False,
                   enable_asserts=False, num_devices=B)

    xT_d = nc.dram_tensor("xT", [D, T], bf16, kind="ExternalInput")
    kT_d = nc.dram_tensor("kT", [D, S], bf16, kind="ExternalInput")
    va_d = nc.dram_tensor("vaug", [S, H * HW], bf16, kind="ExternalInput")
    wqT_d = nc.dram_tensor("wqT", [D, D], bf16, kind="ExternalInput")
    bq_d = nc.dram_tensor("bqr", [P, DT], f32, kind="ExternalInput")
    woT_d = nc.dram_tensor("woT", [D, D], bf16, kind="ExternalInput")
    bo_d = nc.dram_tensor("bor", [P, DT], f32, kind="ExternalInput")
    outT_d = nc.dram_tensor("outT", [D, T], f32, kind="ExternalOutput")

    EXP = mybir.ActivationFunctionType.Exp

    with tile.TileContext(nc) as tc:
        with (
            tc.tile_pool(name="const", bufs=1) as cp,
            tc.tile_pool(name="work", bufs=2) as wp,
            tc.tile_pool(name="psum_mm", bufs=2, space="PSUM") as mmp,
            tc.tile_pool(name="psum_pv", bufs=2, space="PSUM") as pvp,
        ):
            # ---- persistent SBUF loads -------------------------------------
            def load1(dram, cols, j, tagbase, dt=bf16):
                t = cp.tile([P, cols], dt, name=f"{tagbase}{j}",
                            tag=f"{tagbase}{j}")
                nc.sync.dma_start(t[:], dram[j * P:(j + 1) * P, :])
                return t

            bq_sb = cp.tile([P, DT], f32, name="bq_sb", tag="bq_sb")
            nc.sync.dma_start(bq_sb[:], bq_d[:, :])
            # order matters: q_proj needs all of xT+wqT; pair 0 additionally
            # needs kT[0] and the vaug chunks — issue those before the rest.
            xT_sb = [load1(xT_d, T, j, "xTs") for j in range(DT)]
            kT_sb = {0: load1(kT_d, S, 0, "kTs")}
            va_sb = {0: load1(va_d, H * HW, 0, "vas")}
            wqT_sb = [load1(wqT_d, D, j, "wqTs") for j in range(DT)]
            va_sb.update({c: load1(va_d, H * HW, c, "vas") for c in (1, 2, 3)})
            kT_sb.update({j: load1(kT_d, S, j, "kTs") for j in range(1, DT)})
            va_sb.update({c: load1(va_d, H * HW, c, "vas")
                          for c in range(4, ST)})
            woT_sb = [load1(woT_d, D, j, "woTs") for j in range(DT)]
            bo_sb = cp.tile([P, DT], f32, name="bo_sb", tag="bo_sb")
            nc.sync.dma_start(bo_sb[:], bo_d[:, :])

            qT_sb = [cp.tile([P, T], bf16, name=f"qTs{j}", tag=f"qTs{j}")
                     for j in range(DT)]
            aT_sb = [cp.tile([P, T], bf16, name=f"aTs{j}", tag=f"aTs{j}")
                     for j in range(DT)]

            # ---- phase 1: q projection  qT[e,t] ----------------------------
            for j in range(DT):
                ps = mmp.tile([P, T], f32, name=f"qp{j}", tag="mm")
                for tch in range(2):
                    tsl = slice(tch * 512, (tch + 1) * 512)
                    for dt_i in range(DT):
                        nc.tensor.matmul(
                            ps[:, tsl],
                            lhsT=wqT_sb[dt_i][:, j * P:(j + 1) * P],
                            rhs=xT_sb[dt_i][:, tsl],
                            start=(dt_i == 0), stop=(dt_i == DT - 1),
                        )
                nc.vector.tensor_scalar_add(qT_sb[j][:, :], ps[:, :],
                                            bq_sb[:, j:j + 1])

            # ---- phase 2: attention, one head pair per e-tile --------------
            for j in range(NPAIR):
                pv = [pvp.tile([P, T], f32, name=f"pv{j}_{a}", tag="pv")
                      for a in range(2)]
                for c in range(ST):
                    csl = slice(c * P, (c + 1) * P)
                    # two K=64 score matmuls per head, heads at partition
                    # bases 0/64 -> disjoint PE row groups; emit adjacently
                    # so they execute concurrently in the array.
                    sc = [mmp.tile([P, T], f32, name=f"sc{j}_{c}_{a}",
                                   tag="mm") for a in range(2)]
                    for a in range(2):
                        rows = slice(a * Dh, (a + 1) * Dh)
                        for tch in range(2):
                            tsl = slice(tch * 512, (tch + 1) * 512)
                            nc.tensor.matmul(
                                sc[a][:, tsl],
                                lhsT=kT_sb[j][rows, csl],
                                rhs=qT_sb[j][rows, tsl],
                                start=True, stop=True,
                            )
                    pt = [None, None]
                    for a in range(2):
                        pt[a] = wp.tile([P, T], bf16, name=f"pt{j}_{c}_{a}",
                                        tag="pt", bufs=8)
                        nc.scalar.activation(pt[a][:, :], sc[a][:, :], EXP)
                    for a in range(2):
                        h = 2 * j + a
                        for tch in range(2):
                            tsl = slice(tch * 512, (tch + 1) * 512)
                            nc.tensor.matmul(
                                pv[a][0:HW, tsl],
                                lhsT=va_sb[c][:, h * HW:(h + 1) * HW],
                                rhs=pt[a][:, tsl],
                                start=(c == 0), stop=(c == ST - 1),
                            )
                # evict PSUM fast, then normalize out of SBUF -> attnT (bf16)
                for a in range(2):
                    pvsb = wp.tile([HW, T], f32, name=f"pvsb{j}_{a}",
                                   tag="pvsb", bufs=3)
                    nc.vector.tensor_copy(pvsb[:, :], pv[a][0:HW, :])
                    # reciprocal of the [1,1024] denom row is ~6.5us (one
                    # lane, multi-pass); reshape to [64,16] via SB->SB DMA
                    # so all 64 lanes work -> ~0.4us.
                    dsm = wp.tile([Dh, 16], f32, name=f"ds{j}_{a}",
                                  tag="dsm", bufs=4)
                    nc.sync.dma_start(dsm[:, :], pvsb[Dh:Dh + 1, :])
                    nc.vector.reciprocal(dsm[:, :], dsm[:, :])
                    rrow = wp.tile([1, T], f32, name=f"rr{j}_{a}", tag="rrow",
                                   bufs=4)
                    nc.sync.dma_start(rrow[:, :], dsm[:, :])
                    nrm = wp.tile([Dh, T], f32, name=f"nr{j}_{a}", tag="nrm",
                                  bufs=4)
                    nc.gpsimd.partition_broadcast(nrm[:, :], rrow[0:1, :])
                    nc.vector.tensor_mul(
                        aT_sb[j][a * Dh:(a + 1) * Dh, :],
                        pvsb[0:Dh, :], nrm[:, :])

            # ---- phase 3: out projection  outT[f,t] ------------------------
            for fj in range(DT):
                ps = mmp.tile([P, T], f32, name=f"op{fj}", tag="mm")
                for tch in range(2):
                    tsl = slice(tch * 512, (tch + 1) * 512)
                    for et in range(DT):
                        nc.tensor.matmul(
                            ps[:, tsl],
                            lhsT=woT_sb[et][:, fj * P:(fj + 1) * P],
                            rhs=aT_sb[et][:, tsl],
                            start=(et == 0), stop=(et == DT - 1),
                        )
                ost = wp.tile([P, T], f32, name=f"ost{fj}", tag="ost", bufs=3)
                nc.vector.tensor_scalar_add(ost[:, :], ps[:, :],
                                            bo_sb[:, fj:fj + 1])
                nc.sync.dma_start(outT_d[fj * P:(fj + 1) * P, :], ost[:, :])

    nc.compile()
    return nc


def prep_inputs(x, k, v, wq, bq, wo, bo):
    """Host-side shard + layout prep. Returns per-core in_maps."""
    x = np.asarray(x, np.float32)
    k = np.asarray(k, np.float32)
    v = np.asarray(v, np.float32)
    wq = np.asarray(wq, np.float32)
    bq = np.asarray(bq, np.float32)
    wo = np.asarray(wo, np.float32)
    bo = np.asarray(bo, np.float32)

    wqT = np.ascontiguousarray((wq * SCALE).T).astype(BF16)       # [d, e]
    bqr = np.ascontiguousarray((bq * SCALE).reshape(DT, P).T)     # [P, DT]
    woT = np.ascontiguousarray(wo.T).astype(BF16)                 # [e, f]
    bor = np.ascontiguousarray(bo.reshape(DT, P).T)               # [P, DT]

    in_maps = []
    for b in range(x.shape[0]):
        xT = np.ascontiguousarray(x[b].T).astype(BF16)            # [D, T]
        kT = np.zeros((D, S), BF16)
        kT[:, :S0] = k[b].T.astype(BF16)
        vaug = np.zeros((S, H * HW), BF16)
        vb = v[b].astype(BF16)
        for h in range(H):
            vaug[:S0, h * HW:h * HW + Dh] = vb[:, h * Dh:(h + 1) * Dh]
            vaug[:S0, h * HW + Dh] = BF16(1.0)
        in_maps.append({
            "xT": xT, "kT": kT, "vaug": np.ascontiguousarray(vaug),
            "wqT": wqT, "bqr": bqr, "woT": woT, "bor": bor,
        })
    return in_maps


_NC_CACHE = {}


def kernel(x, k, v, wq, bq, wo, bo, _trace=False):
    if "nc" not in _NC_CACHE:
        _NC_CACHE["nc"] = build_bass()
    nc = _NC_CACHE["nc"]
    in_maps = prep_inputs(x, k, v, wq, bq, wo, bo)
    res = bass_utils.run_bass_kernel_spmd(
        nc, in_maps, core_ids=list(range(B)), trace=_trace)
    _NC_CACHE["last_result"] = res
    out = np.stack([np.ascontiguousarray(r["outT"].T) for r in res.results])
    return out
